# revision 12
# baseline (speedup 1.0000x reference)
"""Trainium2 Bass kernel for CubicSplineAutoregressiveSubsetTransform2d.

Computes, per element (B,C,H,W), a monotone cubic Hermite spline (nsf
cubic_spline forward) parameterized by 34 per-element params
(16 widths, 16 heights, 2 derivs), applied to two inputs x_lower/x_upper.

Key algorithmic trick: the spline is monotone increasing, so instead of
searchsorted + gather we use the telescoping identity

    z(x) = sum_k [ a_k s_k^3 + b_k s_k^2 + c_k s_k ],
    s_k  = clamp(x - W_k, 0, w_k)

where full bins contribute exactly h_k (spline interpolates knots) and the
partial bin contributes the local cubic. No masks, no gathers.

Sharding: pure data-parallel over batch dim across 8 NeuronCores.
"""

import os
import sys

import numpy as np

for _p in ("/opt/trn_rl_repo",):
    if _p not in sys.path:
        sys.path.insert(0, _p)

import concourse.bass as bass
import concourse.bacc as bacc
import concourse.mybir as mybir
from concourse import tile
from concourse.bass_utils import run_bass_kernel_spmd

F32 = mybir.dt.float32
AX = mybir.AxisListType
OP = mybir.AluOpType
ACT = mybir.ActivationFunctionType

B, C, H, W, K = 32, 3, 128, 128, 16
N_CORES = 8
NC_ELEMS = (B // N_CORES) * C * H * W  # 196608 per core
MIN_BIN = 1e-3
SCALE = 1.0 - MIN_BIN * K  # 0.984


def build_program(n_elems: int, S: int = 96):
    """Build the SPMD Bass program for one core processing n_elems elements."""
    P = 128
    per_tile = P * S
    assert n_elems % per_tile == 0
    T = n_elems // per_tile

    nc = bacc.Bacc()
    xl_d = nc.dram_tensor("x_lower", [n_elems], F32, kind="ExternalInput")
    xu_d = nc.dram_tensor("x_upper", [n_elems], F32, kind="ExternalInput")
    pp_d = nc.dram_tensor("elementwise_params", [n_elems, 2 * K + 2], F32,
                          kind="ExternalInput")
    zl_d = nc.dram_tensor("z_lower", [n_elems], F32, kind="ExternalOutput")
    zu_d = nc.dram_tensor("z_upper", [n_elems], F32, kind="ExternalOutput")

    pr = pp_d[:].rearrange("(t p s) k -> t p s k", p=P, s=S)
    xlr = xl_d[:].rearrange("(t p s) -> t p s", p=P, s=S)
    xur = xu_d[:].rearrange("(t p s) -> t p s", p=P, s=S)
    zlr = zl_d[:].rearrange("(t p s) -> t p s", p=P, s=S)
    zur = zu_d[:].rearrange("(t p s) -> t p s", p=P, s=S)

    with tile.TileContext(nc) as tc:
        with tc.tile_pool(name="cst", bufs=1) as cst, \
             tc.tile_pool(name="io", bufs=2) as io, \
             tc.tile_pool(name="wk", bufs=1) as wk, \
             tc.tile_pool(name="ac", bufs=2) as ac:
            # segment mask for the in-tile cumsum scan: 0 at k=0, 1 elsewhere
            segm = cst.tile([P, S, K], F32, tag="segm")
            nc.vector.memset(segm[:], 1.0)
            nc.vector.memset(segm[:, :, 0:1], 0.0)
            for t in range(T):
                # param load issued from ACT: its consumers (exp/tanh) are
                # also on ACT, so the WAR dep is program-order and the DMA
                # carries only the single DMA-WAW semaphore wait (walrus
                # DIRECT2D DMAs support exactly one sync wait).
                raw = io.tile([P, S, 34], F32, tag="raw")
                nc.scalar.dma_start(out=raw[:], in_=pr[t])
                # x loads also on ACT, bounced through an ACT copy so the
                # load's only consumer is same-engine (program order) and the
                # DMA keeps a single sem wait (the DMA-WAW).
                xlt = io.tile([P, S], F32, tag="xl")
                nc.scalar.dma_start(out=xlt[:], in_=xlr[t])
                xut = io.tile([P, S], F32, tag="xu")
                nc.scalar.dma_start(out=xut[:], in_=xur[t])
                xlc = wk.tile([P, S], F32, tag="xlc")
                nc.scalar.copy(xlc[:], xlt[:])
                xuc = wk.tile([P, S], F32, tag="xuc")
                nc.scalar.copy(xuc[:], xut[:])

                # --- softmax-scaled widths & heights -------------------------
                # one exp over both uw and uh (single consumer of raw slice)
                ewh = ac.tile([P, S, 2 * K], F32, tag="ewh")
                nc.scalar.activation(ewh[:], raw[:, :, 0:2 * K], ACT.Exp)
                ew = ewh[:, :, 0:K]
                eh = ewh[:, :, K:2 * K]

                Sw = wk.tile([P, S], F32, tag="Sw")
                nc.vector.reduce_sum(Sw[:], ew, axis=AX.X)
                Sh = wk.tile([P, S], F32, tag="Sh")
                nc.vector.reduce_sum(Sh[:], eh, axis=AX.X)
                rSw = wk.tile([P, S], F32, tag="rSw")
                nc.vector.reciprocal(rSw[:], Sw[:])
                rSh = wk.tile([P, S], F32, tag="rSh")
                nc.vector.reciprocal(rSh[:], Sh[:])

                rSw_b = rSw[:].unsqueeze(2).broadcast_to([P, S, K])
                rSh_b = rSh[:].unsqueeze(2).broadcast_to([P, S, K])

                wt = wk.tile([P, S, K], F32, tag="wt")  # true widths
                nc.vector.tensor_tensor(wt[:], ew, rSw_b, OP.mult)
                nc.vector.tensor_scalar(wt[:], wt[:], SCALE, MIN_BIN,
                                        OP.mult, OP.add)
                ht = wk.tile([P, S, K], F32, tag="ht")  # true heights
                nc.vector.tensor_tensor(ht[:], eh, rSh_b, OP.mult)
                nc.vector.tensor_scalar(ht[:], ht[:], SCALE, MIN_BIN,
                                        OP.mult, OP.add)

                # --- cumulative widths: segmented scan ----------------------
                # state = segm*state + w  ->  per-slot inclusive cumsum
                cw = wk.tile([P, S, K], F32, tag="cw")
                nc.vector.tensor_tensor_scan(
                    cw[:].rearrange("p s k -> p (s k)"),
                    segm[:].rearrange("p s k -> p (s k)"),
                    wt[:].rearrange("p s k -> p (s k)"),
                    0.0, OP.mult, OP.add)

                # --- slopes --------------------------------------------------
                rw = wk.tile([P, S, K], F32, tag="rw")
                nc.vector.reciprocal_approx_fast(rw[:], wt[:])
                st_ = wk.tile([P, S, K], F32, tag="st")
                nc.vector.tensor_tensor(st_[:], ht[:], rw[:], OP.mult)

                # --- interior derivatives (Fritsch-Carlson style) ------------
                # delta_i = min(2*min(s_{i-1},s_i),
                #               (w_i s_{i-1} + w_{i-1} s_i)/(w_{i-1}+w_i))
                sL = st_[:, :, 0:K - 1]
                sR = st_[:, :, 1:K]
                wL = wt[:, :, 0:K - 1]
                wR = wt[:, :, 1:K]

                m1 = wk.tile([P, S, K - 1], F32, tag="m1")
                nc.vector.tensor_tensor(m1[:], sL, sR, OP.min)
                nc.vector.tensor_scalar_mul(m1[:], m1[:], 2.0)

                t1 = wk.tile([P, S, K - 1], F32, tag="t1")
                nc.vector.tensor_tensor(t1[:], wR, sL, OP.mult)
                t2 = wk.tile([P, S, K - 1], F32, tag="t2")
                nc.vector.tensor_tensor(t2[:], wL, sR, OP.mult)
                nc.vector.tensor_tensor(t1[:], t1[:], t2[:], OP.add)  # num
                nc.vector.tensor_tensor(t2[:], wL, wR, OP.add)        # den
                rden = wk.tile([P, S, K - 1], F32, tag="rden")
                nc.vector.reciprocal_approx_fast(rden[:], t2[:])
                nc.vector.tensor_tensor(t1[:], t1[:], rden[:], OP.mult)  # m2*2

                dlt = wk.tile([P, S, K + 1], F32, tag="dlt")
                nc.vector.tensor_tensor(dlt[:, :, 1:K], m1[:], t1[:], OP.min)

                # --- boundary derivatives -----------------------------------
                # d_0 = sigmoid(udl)*3*s_0 ; sigmoid(u) = 0.5*tanh(u/2)+0.5
                e01 = wk.tile([P, S, 2], F32, tag="e01")
                nc.scalar.activation(e01[:], raw[:, :, 2 * K:2 * K + 2],
                                     ACT.Tanh, scale=0.5)
                nc.vector.tensor_scalar(e01[:], e01[:], 1.5, 1.5, OP.mult, OP.add)
                nc.vector.tensor_tensor(dlt[:, :, 0:1], e01[:, :, 0:1],
                                        st_[:, :, 0:1], OP.mult)
                nc.vector.tensor_tensor(dlt[:, :, K:K + 1], e01[:, :, 1:2],
                                        st_[:, :, K - 1:K], OP.mult)

                # --- cubic coefficients per bin ------------------------------
                D0 = dlt[:, :, 0:K]
                D1 = dlt[:, :, 1:K + 1]
                ds = wk.tile([P, S, K], F32, tag="ds")
                nc.vector.tensor_tensor(ds[:], D0, D1, OP.add)
                # contribution = sg*(D0 + u*(B' - A'*u)), u = sg/w
                # A' = 2s - ds ; B' = 3s - D0 - ds
                aN = wk.tile([P, S, K], F32, tag="aN")
                nc.vector.scalar_tensor_tensor(aN[:], st_[:], 2.0, ds[:],
                                               OP.mult, OP.subtract)
                bc = wk.tile([P, S, K], F32, tag="bc")
                nc.vector.scalar_tensor_tensor(bc[:], st_[:], 3.0, D0,
                                               OP.mult, OP.subtract)
                nc.vector.tensor_tensor(bc[:], bc[:], ds[:], OP.subtract)

                # --- evaluate for both x tensors -----------------------------
                for xt, zr in ((xlc, zlr), (xuc, zur)):
                    x_b = xt[:].unsqueeze(2).broadcast_to([P, S, K - 1])
                    tt = wk.tile([P, S, K], F32, tag="tt")
                    nc.vector.tensor_copy(tt[:, :, 0:1], xt[:].unsqueeze(2))
                    nc.vector.tensor_tensor(tt[:, :, 1:K], x_b,
                                            cw[:, :, 0:K - 1], OP.subtract)
                    # sigma = clamp(tt, 0, w): relu on ACT, min on GpSimd
                    sr = wk.tile([P, S, K], F32, tag="sr")
                    nc.scalar.activation(sr[:], tt[:], ACT.Relu)
                    sg = wk.tile([P, S, K], F32, tag="sg")
                    nc.vector.tensor_tensor(sg[:], sr[:], wt[:], OP.min)
                    # u = sg/w ; contribution = sg*(D0 + u*(B' - A'*u))
                    uu = wk.tile([P, S, K], F32, tag="uu")
                    nc.vector.tensor_tensor(uu[:], sg[:], rw[:], OP.mult)
                    hv = wk.tile([P, S, K], F32, tag="hv")
                    nc.vector.tensor_tensor(hv[:], aN[:], uu[:], OP.mult)
                    nc.vector.tensor_tensor(hv[:], bc[:], hv[:], OP.subtract)
                    nc.vector.tensor_tensor(hv[:], hv[:], uu[:], OP.mult)
                    nc.vector.tensor_tensor(hv[:], hv[:], D0, OP.add)
                    nc.vector.tensor_tensor(hv[:], hv[:], sg[:], OP.mult)
                    zt = wk.tile([P, S], F32, tag="zt")
                    nc.vector.reduce_sum(zt[:], hv[:], axis=AX.X)
                    nc.vector.tensor_scalar(zt[:], zt[:], 1.0, 0.0,
                                            OP.min, OP.max)
                    # bounce through ACT so the store DMA (issued on ACT) has
                    # a program-order dep on its producer; its single sem
                    # wait is then the DRAM-tensor WAW vs the previous store.
                    zb = io.tile([P, S], F32, tag="zb" + zr.tensor.name)
                    nc.scalar.copy(zb[:], zt[:])
                    nc.scalar.dma_start(out=zr[t], in_=zb[:])
    nc.finalize()
    return nc


_PROGRAM_CACHE = {}


def _get_program(n_elems, S=96):
    key = (n_elems, S)
    if key not in _PROGRAM_CACHE:
        _PROGRAM_CACHE[key] = build_program(n_elems, S)
    return _PROGRAM_CACHE[key]


def kernel(x_lower, x_upper, elementwise_params):
    x_lower = np.ascontiguousarray(x_lower, dtype=np.float32)
    x_upper = np.ascontiguousarray(x_upper, dtype=np.float32)
    elementwise_params = np.ascontiguousarray(elementwise_params,
                                              dtype=np.float32)
    Bb = x_lower.shape[0]
    per = Bb // N_CORES
    n_elems = per * C * H * W

    nc = _get_program(n_elems)
    in_maps = []
    for c in range(N_CORES):
        sl = slice(c * per, (c + 1) * per)
        in_maps.append({
            "x_lower": x_lower[sl].reshape(n_elems),
            "x_upper": x_upper[sl].reshape(n_elems),
            "elementwise_params": elementwise_params[sl].reshape(n_elems, 34),
        })
    res = run_bass_kernel_spmd(nc, in_maps, list(range(N_CORES)))
    zl = np.concatenate([r["z_lower"].reshape(per, C, H, W)
                         for r in res.results], axis=0)
    zu = np.concatenate([r["z_upper"].reshape(per, C, H, W)
                         for r in res.results], axis=0)
    return zl, zu


if __name__ == "__main__":
    rng = np.random.default_rng(0)
    xl = rng.random((B, C, H, W), dtype=np.float32)
    xu = rng.random((B, C, H, W), dtype=np.float32)
    pp = rng.standard_normal((B, C, H, W, 34), dtype=np.float32)
    zl, zu = kernel(x_lower=xl, x_upper=xu, elementwise_params=pp)
    print("ok", zl.shape, zu.shape, zl.min(), zl.max())


# revision 14
# speedup vs baseline: 1.0006x; 1.0006x over previous
"""Trainium2 Bass kernel for CubicSplineAutoregressiveSubsetTransform2d.

Computes, per element (B,C,H,W), a monotone cubic Hermite spline (nsf
cubic_spline forward) parameterized by 34 per-element params
(16 widths, 16 heights, 2 derivs), applied to two inputs x_lower/x_upper.

Key algorithmic trick: the spline is monotone increasing, so instead of
searchsorted + gather we use the telescoping identity

    z(x) = sum_k [ a_k s_k^3 + b_k s_k^2 + c_k s_k ],
    s_k  = clamp(x - W_k, 0, w_k)

where full bins contribute exactly h_k (spline interpolates knots) and the
partial bin contributes the local cubic. No masks, no gathers.

Sharding: pure data-parallel over batch dim across 8 NeuronCores.
"""

import os
import sys

import numpy as np

for _p in ("/opt/trn_rl_repo",):
    if _p not in sys.path:
        sys.path.insert(0, _p)

import concourse.bass as bass
import concourse.bacc as bacc
import concourse.mybir as mybir
from concourse import tile
from concourse.bass_utils import run_bass_kernel_spmd

F32 = mybir.dt.float32
BF16 = mybir.dt.bfloat16
AX = mybir.AxisListType
OP = mybir.AluOpType
ACT = mybir.ActivationFunctionType

B, C, H, W, K = 32, 3, 128, 128, 16
N_CORES = 8
NC_ELEMS = (B // N_CORES) * C * H * W  # 196608 per core
MIN_BIN = 1e-3
SCALE = 1.0 - MIN_BIN * K  # 0.984


def build_program(n_elems: int, S: int = 96):
    """Build the SPMD Bass program for one core processing n_elems elements."""
    P = 128
    per_tile = P * S
    assert n_elems % per_tile == 0
    T = n_elems // per_tile

    nc = bacc.Bacc()
    xl_d = nc.dram_tensor("x_lower", [n_elems], F32, kind="ExternalInput")
    xu_d = nc.dram_tensor("x_upper", [n_elems], F32, kind="ExternalInput")
    pp_d = nc.dram_tensor("elementwise_params", [n_elems, 2 * K + 2], F32,
                          kind="ExternalInput")
    zl_d = nc.dram_tensor("z_lower", [n_elems], F32, kind="ExternalOutput")
    zu_d = nc.dram_tensor("z_upper", [n_elems], F32, kind="ExternalOutput")

    pr = pp_d[:].rearrange("(t p s) k -> t p s k", p=P, s=S)
    xlr = xl_d[:].rearrange("(t p s) -> t p s", p=P, s=S)
    xur = xu_d[:].rearrange("(t p s) -> t p s", p=P, s=S)
    zlr = zl_d[:].rearrange("(t p s) -> t p s", p=P, s=S)
    zur = zu_d[:].rearrange("(t p s) -> t p s", p=P, s=S)

    with tile.TileContext(nc) as tc:
        with tc.tile_pool(name="cst", bufs=1) as cst, \
             tc.tile_pool(name="io", bufs=2) as io, \
             tc.tile_pool(name="wk", bufs=1) as wk, \
             tc.tile_pool(name="ac", bufs=2) as ac:
            # segment mask for the in-tile cumsum scan: 0 at k=0, 1 elsewhere
            segm = cst.tile([P, S, K], F32, tag="segm")
            nc.vector.memset(segm[:], 1.0)
            nc.vector.memset(segm[:, :, 0:1], 0.0)
            for t in range(T):
                # param load issued from ACT: its consumers (exp/tanh) are
                # also on ACT, so the WAR dep is program-order and the DMA
                # carries only the single DMA-WAW semaphore wait (walrus
                # DIRECT2D DMAs support exactly one sync wait).
                raw = io.tile([P, S, 34], F32, tag="raw")
                nc.scalar.dma_start(out=raw[:], in_=pr[t])
                # x loads also on ACT, bounced through an ACT copy so the
                # load's only consumer is same-engine (program order) and the
                # DMA keeps a single sem wait (the DMA-WAW).
                xlt = io.tile([P, S], F32, tag="xl")
                nc.scalar.dma_start(out=xlt[:], in_=xlr[t])
                xut = io.tile([P, S], F32, tag="xu")
                nc.scalar.dma_start(out=xut[:], in_=xur[t])
                xlc = wk.tile([P, S], F32, tag="xlc")
                nc.scalar.copy(xlc[:], xlt[:])
                xuc = wk.tile([P, S], F32, tag="xuc")
                nc.scalar.copy(xuc[:], xut[:])

                # --- softmax-scaled widths & heights -------------------------
                # one exp over both uw and uh (single consumer of raw slice)
                ewh = ac.tile([P, S, 2 * K], F32, tag="ewh")
                nc.scalar.activation(ewh[:], raw[:, :, 0:2 * K], ACT.Exp)
                ew = ewh[:, :, 0:K]
                eh = ewh[:, :, K:2 * K]

                Sw = wk.tile([P, S], F32, tag="Sw")
                nc.vector.reduce_sum(Sw[:], ew, axis=AX.X)
                Sh = wk.tile([P, S], F32, tag="Sh")
                nc.vector.reduce_sum(Sh[:], eh, axis=AX.X)
                rSw = wk.tile([P, S], F32, tag="rSw")
                nc.vector.reciprocal(rSw[:], Sw[:])
                rSh = wk.tile([P, S], F32, tag="rSh")
                nc.vector.reciprocal(rSh[:], Sh[:])

                rSw_b = rSw[:].unsqueeze(2).broadcast_to([P, S, K])
                rSh_b = rSh[:].unsqueeze(2).broadcast_to([P, S, K])

                wt = wk.tile([P, S, K], F32, tag="wt")  # true widths
                nc.vector.tensor_tensor(wt[:], ew, rSw_b, OP.mult)
                nc.vector.tensor_scalar(wt[:], wt[:], SCALE, MIN_BIN,
                                        OP.mult, OP.add)
                ht = wk.tile([P, S, K], F32, tag="ht")  # true heights
                nc.vector.tensor_tensor(ht[:], eh, rSh_b, OP.mult)
                nc.vector.tensor_scalar(ht[:], ht[:], SCALE, MIN_BIN,
                                        OP.mult, OP.add)

                # --- cumulative widths: segmented scan ----------------------
                # state = segm*state + w  ->  per-slot inclusive cumsum
                cw = wk.tile([P, S, K], F32, tag="cw")
                nc.vector.tensor_tensor_scan(
                    cw[:].rearrange("p s k -> p (s k)"),
                    segm[:].rearrange("p s k -> p (s k)"),
                    wt[:].rearrange("p s k -> p (s k)"),
                    0.0, OP.mult, OP.add)

                # --- slopes --------------------------------------------------
                rw = wk.tile([P, S, K], F32, tag="rw")
                nc.vector.reciprocal_approx_fast(rw[:], wt[:])
                st_ = wk.tile([P, S, K], F32, tag="st")
                nc.vector.tensor_tensor(st_[:], ht[:], rw[:], OP.mult)

                # --- interior derivatives (Fritsch-Carlson style) ------------
                # delta_i = min(2*min(s_{i-1},s_i),
                #               (w_i s_{i-1} + w_{i-1} s_i)/(w_{i-1}+w_i))
                sL = st_[:, :, 0:K - 1]
                sR = st_[:, :, 1:K]
                wL = wt[:, :, 0:K - 1]
                wR = wt[:, :, 1:K]

                m1 = wk.tile([P, S, K - 1], F32, tag="m1")
                nc.vector.tensor_tensor(m1[:], sL, sR, OP.min)
                nc.vector.tensor_scalar_mul(m1[:], m1[:], 2.0)

                t1 = wk.tile([P, S, K - 1], F32, tag="t1")
                nc.vector.tensor_tensor(t1[:], wR, sL, OP.mult)
                t2 = wk.tile([P, S, K - 1], F32, tag="t2")
                nc.vector.tensor_tensor(t2[:], wL, sR, OP.mult)
                nc.vector.tensor_tensor(t1[:], t1[:], t2[:], OP.add)  # num
                nc.vector.tensor_tensor(t2[:], wL, wR, OP.add)        # den
                rden = wk.tile([P, S, K - 1], F32, tag="rden")
                nc.vector.reciprocal_approx_fast(rden[:], t2[:])
                nc.vector.tensor_tensor(t1[:], t1[:], rden[:], OP.mult)  # m2*2

                dlt = wk.tile([P, S, K + 1], F32, tag="dlt")
                nc.vector.tensor_tensor(dlt[:, :, 1:K], m1[:], t1[:], OP.min)

                # --- boundary derivatives -----------------------------------
                # d_0 = sigmoid(udl)*3*s_0 ; sigmoid(u) = 0.5*tanh(u/2)+0.5
                e01 = wk.tile([P, S, 2], F32, tag="e01")
                nc.scalar.activation(e01[:], raw[:, :, 2 * K:2 * K + 2],
                                     ACT.Tanh, scale=0.5)
                nc.vector.tensor_scalar(e01[:], e01[:], 1.5, 1.5, OP.mult, OP.add)
                nc.vector.tensor_tensor(dlt[:, :, 0:1], e01[:, :, 0:1],
                                        st_[:, :, 0:1], OP.mult)
                nc.vector.tensor_tensor(dlt[:, :, K:K + 1], e01[:, :, 1:2],
                                        st_[:, :, K - 1:K], OP.mult)

                # --- cubic coefficients per bin ------------------------------
                D0 = dlt[:, :, 0:K]
                D1 = dlt[:, :, 1:K + 1]
                ds = wk.tile([P, S, K], F32, tag="ds")
                nc.vector.tensor_tensor(ds[:], D0, D1, OP.add)
                # contribution = sg*(D0 + u*(B' - A'*u)), u = sg/w
                # A' = 2s - ds ; B' = 3s - D0 - ds
                aN = wk.tile([P, S, K], F32, tag="aN")
                nc.vector.scalar_tensor_tensor(aN[:], st_[:], 2.0, ds[:],
                                               OP.mult, OP.subtract)
                bc = wk.tile([P, S, K], F32, tag="bc")
                nc.vector.scalar_tensor_tensor(bc[:], st_[:], 3.0, D0,
                                               OP.mult, OP.subtract)
                nc.vector.tensor_tensor(bc[:], bc[:], ds[:], OP.subtract)

                # --- evaluate for both x tensors -----------------------------
                for xt, zr in ((xlc, zlr), (xuc, zur)):
                    x_b = xt[:].unsqueeze(2).broadcast_to([P, S, K - 1])
                    tt = wk.tile([P, S, K], F32, tag="tt")
                    nc.vector.tensor_copy(tt[:, :, 0:1], xt[:].unsqueeze(2))
                    nc.vector.tensor_tensor(tt[:, :, 1:K], x_b,
                                            cw[:, :, 0:K - 1], OP.subtract)
                    # sigma = clamp(tt, 0, w): relu on ACT, rest bf16@2x
                    sr = wk.tile([P, S, K], F32, tag="sr")
                    nc.scalar.activation(sr[:], tt[:], ACT.Relu)
                    sg = wk.tile([P, S, K], F32, tag="sg")
                    nc.vector.tensor_tensor(sg[:], sr[:], wt[:], OP.min)
                    # u = sg/w ; contribution = sg*(D0 + u*(B' - A'*u))
                    uu = wk.tile([P, S, K], F32, tag="uu")
                    nc.vector.tensor_tensor(uu[:], sg[:], rw[:], OP.mult)
                    hv = wk.tile([P, S, K], F32, tag="hv")
                    nc.vector.tensor_tensor(hv[:], aN[:], uu[:], OP.mult)
                    nc.vector.tensor_tensor(hv[:], bc[:], hv[:], OP.subtract)
                    nc.vector.tensor_tensor(hv[:], hv[:], uu[:], OP.mult)
                    nc.vector.tensor_tensor(hv[:], hv[:], D0, OP.add)
                    nc.vector.tensor_tensor(hv[:], hv[:], sg[:], OP.mult)
                    zt = wk.tile([P, S], F32, tag="zt")
                    nc.vector.reduce_sum(zt[:], hv[:], axis=AX.X)
                    nc.vector.tensor_scalar(zt[:], zt[:], 1.0, 0.0,
                                            OP.min, OP.max)
                    # bounce through ACT so the store DMA (issued on ACT) has
                    # a program-order dep on its producer; its single sem
                    # wait is then the DRAM-tensor WAW vs the previous store.
                    zb = io.tile([P, S], F32, tag="zb" + zr.tensor.name)
                    nc.scalar.copy(zb[:], zt[:])
                    nc.scalar.dma_start(out=zr[t], in_=zb[:])
    nc.finalize()
    return nc


_PROGRAM_CACHE = {}


def _get_program(n_elems, S=96):
    key = (n_elems, S)
    if key not in _PROGRAM_CACHE:
        _PROGRAM_CACHE[key] = build_program(n_elems, S)
    return _PROGRAM_CACHE[key]


def kernel(x_lower, x_upper, elementwise_params):
    x_lower = np.ascontiguousarray(x_lower, dtype=np.float32)
    x_upper = np.ascontiguousarray(x_upper, dtype=np.float32)
    elementwise_params = np.ascontiguousarray(elementwise_params,
                                              dtype=np.float32)
    Bb = x_lower.shape[0]
    per = Bb // N_CORES
    n_elems = per * C * H * W

    nc = _get_program(n_elems)
    in_maps = []
    for c in range(N_CORES):
        sl = slice(c * per, (c + 1) * per)
        in_maps.append({
            "x_lower": x_lower[sl].reshape(n_elems),
            "x_upper": x_upper[sl].reshape(n_elems),
            "elementwise_params": elementwise_params[sl].reshape(n_elems, 34),
        })
    res = run_bass_kernel_spmd(nc, in_maps, list(range(N_CORES)))
    zl = np.concatenate([r["z_lower"].reshape(per, C, H, W)
                         for r in res.results], axis=0)
    zu = np.concatenate([r["z_upper"].reshape(per, C, H, W)
                         for r in res.results], axis=0)
    return zl, zu


if __name__ == "__main__":
    rng = np.random.default_rng(0)
    xl = rng.random((B, C, H, W), dtype=np.float32)
    xu = rng.random((B, C, H, W), dtype=np.float32)
    pp = rng.standard_normal((B, C, H, W, 34), dtype=np.float32)
    zl, zu = kernel(x_lower=xl, x_upper=xu, elementwise_params=pp)
    print("ok", zl.shape, zu.shape, zl.min(), zl.max())


# revision 15
# speedup vs baseline: 1.0133x; 1.0127x over previous
"""Trainium2 Bass kernel for CubicSplineAutoregressiveSubsetTransform2d.

Computes, per element (B,C,H,W), a monotone cubic Hermite spline (nsf
cubic_spline forward) parameterized by 34 per-element params
(16 widths, 16 heights, 2 derivs), applied to two inputs x_lower/x_upper.

Key algorithmic trick: the spline is monotone increasing, so instead of
searchsorted + gather we use the telescoping identity

    z(x) = sum_k [ a_k s_k^3 + b_k s_k^2 + c_k s_k ],
    s_k  = clamp(x - W_k, 0, w_k)

where full bins contribute exactly h_k (spline interpolates knots) and the
partial bin contributes the local cubic. No masks, no gathers.

Sharding: pure data-parallel over batch dim across 8 NeuronCores.
"""

import os
import sys

import numpy as np

for _p in ("/opt/trn_rl_repo",):
    if _p not in sys.path:
        sys.path.insert(0, _p)

import concourse.bass as bass
import concourse.bacc as bacc
import concourse.mybir as mybir
from concourse import tile
from concourse.bass_utils import run_bass_kernel_spmd

F32 = mybir.dt.float32
BF16 = mybir.dt.bfloat16
AX = mybir.AxisListType
OP = mybir.AluOpType
ACT = mybir.ActivationFunctionType

B, C, H, W, K = 32, 3, 128, 128, 16
N_CORES = 8
NC_ELEMS = (B // N_CORES) * C * H * W  # 196608 per core
MIN_BIN = 1e-3
SCALE = 1.0 - MIN_BIN * K  # 0.984


def build_program(n_elems: int, S: int = 96):
    """Build the SPMD Bass program for one core processing n_elems elements."""
    P = 128
    per_tile = P * S
    assert n_elems % per_tile == 0
    T = n_elems // per_tile

    nc = bacc.Bacc()
    xl_d = nc.dram_tensor("x_lower", [n_elems], F32, kind="ExternalInput")
    xu_d = nc.dram_tensor("x_upper", [n_elems], F32, kind="ExternalInput")
    pp_d = nc.dram_tensor("elementwise_params", [n_elems, 2 * K + 2], F32,
                          kind="ExternalInput")
    zl_d = nc.dram_tensor("z_lower", [n_elems], F32, kind="ExternalOutput")
    zu_d = nc.dram_tensor("z_upper", [n_elems], F32, kind="ExternalOutput")

    pr = pp_d[:].rearrange("(t p s) k -> t p s k", p=P, s=S)
    xlr = xl_d[:].rearrange("(t p s) -> t p s", p=P, s=S)
    xur = xu_d[:].rearrange("(t p s) -> t p s", p=P, s=S)
    zlr = zl_d[:].rearrange("(t p s) -> t p s", p=P, s=S)
    zur = zu_d[:].rearrange("(t p s) -> t p s", p=P, s=S)

    with tile.TileContext(nc) as tc:
        with tc.tile_pool(name="cst", bufs=1) as cst, \
             tc.tile_pool(name="io", bufs=2) as io, \
             tc.tile_pool(name="wk", bufs=1) as wk, \
             tc.tile_pool(name="ac", bufs=2) as ac:
            # segment mask for the in-tile cumsum scan: 0 at k=0, 1 elsewhere
            segm = cst.tile([P, S, K], F32, tag="segm")
            nc.vector.memset(segm[:], 1.0)
            nc.vector.memset(segm[:, :, 0:1], 0.0)
            for t in range(T):
                # param load issued from ACT: its consumers (exp/tanh) are
                # also on ACT, so the WAR dep is program-order and the DMA
                # carries only the single DMA-WAW semaphore wait (walrus
                # DIRECT2D DMAs support exactly one sync wait).
                raw = io.tile([P, S, 34], F32, tag="raw")
                nc.scalar.dma_start(out=raw[:], in_=pr[t])
                # x loads also on ACT, bounced through an ACT copy so the
                # load's only consumer is same-engine (program order) and the
                # DMA keeps a single sem wait (the DMA-WAW).
                xlt = io.tile([P, S], F32, tag="xl")
                nc.scalar.dma_start(out=xlt[:], in_=xlr[t])
                xut = io.tile([P, S], F32, tag="xu")
                nc.scalar.dma_start(out=xut[:], in_=xur[t])
                xlc = wk.tile([P, S], F32, tag="xlc")
                nc.scalar.copy(xlc[:], xlt[:])
                xuc = wk.tile([P, S], F32, tag="xuc")
                nc.scalar.copy(xuc[:], xut[:])

                # --- softmax-scaled widths & heights -------------------------
                # one exp over both uw and uh (single consumer of raw slice)
                ewh = ac.tile([P, S, 2 * K], F32, tag="ewh")
                nc.scalar.activation(ewh[:], raw[:, :, 0:2 * K], ACT.Exp)
                ew = ewh[:, :, 0:K]
                eh = ewh[:, :, K:2 * K]

                Sw = wk.tile([P, S], F32, tag="Sw")
                nc.vector.reduce_sum(Sw[:], ew, axis=AX.X)
                Sh = wk.tile([P, S], F32, tag="Sh")
                nc.vector.reduce_sum(Sh[:], eh, axis=AX.X)
                rSw = wk.tile([P, S], F32, tag="rSw")
                nc.vector.reciprocal_approx_fast(rSw[:], Sw[:])
                rSh = wk.tile([P, S], F32, tag="rSh")
                nc.vector.reciprocal_approx_fast(rSh[:], Sh[:])

                rSw_b = rSw[:].unsqueeze(2).broadcast_to([P, S, K])
                rSh_b = rSh[:].unsqueeze(2).broadcast_to([P, S, K])

                wt = wk.tile([P, S, K], F32, tag="wt")  # true widths
                nc.vector.tensor_tensor(wt[:], ew, rSw_b, OP.mult)
                nc.vector.tensor_scalar(wt[:], wt[:], SCALE, MIN_BIN,
                                        OP.mult, OP.add)
                ht = wk.tile([P, S, K], F32, tag="ht")  # true heights
                nc.vector.tensor_tensor(ht[:], eh, rSh_b, OP.mult)
                nc.vector.tensor_scalar(ht[:], ht[:], SCALE, MIN_BIN,
                                        OP.mult, OP.add)

                # --- cumulative widths: segmented scan ----------------------
                # state = segm*state + w  ->  per-slot inclusive cumsum
                cw = wk.tile([P, S, K], F32, tag="cw")
                nc.vector.tensor_tensor_scan(
                    cw[:].rearrange("p s k -> p (s k)"),
                    segm[:].rearrange("p s k -> p (s k)"),
                    wt[:].rearrange("p s k -> p (s k)"),
                    0.0, OP.mult, OP.add)

                # --- slopes --------------------------------------------------
                rw = wk.tile([P, S, K], F32, tag="rw")
                nc.vector.reciprocal_approx_fast(rw[:], wt[:])
                st_ = wk.tile([P, S, K], F32, tag="st")
                nc.vector.tensor_tensor(st_[:], ht[:], rw[:], OP.mult)

                # --- interior derivatives (Fritsch-Carlson style) ------------
                # delta_i = min(2*min(s_{i-1},s_i),
                #               (w_i s_{i-1} + w_{i-1} s_i)/(w_{i-1}+w_i))
                sL = st_[:, :, 0:K - 1]
                sR = st_[:, :, 1:K]
                wL = wt[:, :, 0:K - 1]
                wR = wt[:, :, 1:K]

                m1 = wk.tile([P, S, K - 1], F32, tag="m1")
                nc.vector.tensor_tensor(m1[:], sL, sR, OP.min)
                nc.vector.tensor_scalar_mul(m1[:], m1[:], 2.0)

                t1 = wk.tile([P, S, K - 1], F32, tag="t1")
                nc.vector.tensor_tensor(t1[:], wR, sL, OP.mult)
                t2 = wk.tile([P, S, K - 1], F32, tag="t2")
                nc.vector.tensor_tensor(t2[:], wL, sR, OP.mult)
                nc.vector.tensor_tensor(t1[:], t1[:], t2[:], OP.add)  # num
                nc.vector.tensor_tensor(t2[:], wL, wR, OP.add)        # den
                rden = wk.tile([P, S, K - 1], F32, tag="rden")
                nc.vector.reciprocal_approx_fast(rden[:], t2[:])
                nc.vector.tensor_tensor(t1[:], t1[:], rden[:], OP.mult)  # m2*2

                dlt = wk.tile([P, S, K + 1], F32, tag="dlt")
                nc.vector.tensor_tensor(dlt[:, :, 1:K], m1[:], t1[:], OP.min)

                # --- boundary derivatives -----------------------------------
                # d_0 = sigmoid(udl)*3*s_0 ; sigmoid(u) = 0.5*tanh(u/2)+0.5
                e01 = wk.tile([P, S, 2], F32, tag="e01")
                nc.scalar.activation(e01[:], raw[:, :, 2 * K:2 * K + 2],
                                     ACT.Tanh, scale=0.5)
                nc.vector.tensor_scalar(e01[:], e01[:], 1.5, 1.5, OP.mult, OP.add)
                nc.vector.tensor_tensor(dlt[:, :, 0:1], e01[:, :, 0:1],
                                        st_[:, :, 0:1], OP.mult)
                nc.vector.tensor_tensor(dlt[:, :, K:K + 1], e01[:, :, 1:2],
                                        st_[:, :, K - 1:K], OP.mult)

                # --- cubic coefficients per bin ------------------------------
                D0 = dlt[:, :, 0:K]
                D1 = dlt[:, :, 1:K + 1]
                ds = wk.tile([P, S, K], F32, tag="ds")
                nc.vector.tensor_tensor(ds[:], D0, D1, OP.add)
                # contribution = sg*(D0 + u*(B' - A'*u)), u = sg/w
                # A' = 2s - ds ; B' = 3s - D0 - ds
                aN = wk.tile([P, S, K], F32, tag="aN")
                nc.vector.scalar_tensor_tensor(aN[:], st_[:], 2.0, ds[:],
                                               OP.mult, OP.subtract)
                bc = wk.tile([P, S, K], F32, tag="bc")
                nc.vector.scalar_tensor_tensor(bc[:], st_[:], 3.0, D0,
                                               OP.mult, OP.subtract)
                nc.vector.tensor_tensor(bc[:], bc[:], ds[:], OP.subtract)

                # --- evaluate for both x tensors -----------------------------
                for xt, zr in ((xlc, zlr), (xuc, zur)):
                    x_b = xt[:].unsqueeze(2).broadcast_to([P, S, K - 1])
                    tt = wk.tile([P, S, K], F32, tag="tt")
                    nc.vector.tensor_copy(tt[:, :, 0:1], xt[:].unsqueeze(2))
                    nc.vector.tensor_tensor(tt[:, :, 1:K], x_b,
                                            cw[:, :, 0:K - 1], OP.subtract)
                    # sigma = clamp(tt, 0, w): relu on ACT, rest bf16@2x
                    sr = wk.tile([P, S, K], F32, tag="sr")
                    nc.scalar.activation(sr[:], tt[:], ACT.Relu)
                    sg = wk.tile([P, S, K], F32, tag="sg")
                    nc.vector.tensor_tensor(sg[:], sr[:], wt[:], OP.min)
                    # u = sg/w ; contribution = sg*(D0 + u*(B' - A'*u))
                    uu = wk.tile([P, S, K], F32, tag="uu")
                    nc.vector.tensor_tensor(uu[:], sg[:], rw[:], OP.mult)
                    hv = wk.tile([P, S, K], F32, tag="hv")
                    nc.vector.tensor_tensor(hv[:], aN[:], uu[:], OP.mult)
                    nc.vector.tensor_tensor(hv[:], bc[:], hv[:], OP.subtract)
                    nc.vector.tensor_tensor(hv[:], hv[:], uu[:], OP.mult)
                    nc.vector.tensor_tensor(hv[:], hv[:], D0, OP.add)
                    nc.vector.tensor_tensor(hv[:], hv[:], sg[:], OP.mult)
                    zt = wk.tile([P, S], F32, tag="zt")
                    nc.vector.reduce_sum(zt[:], hv[:], axis=AX.X)
                    nc.vector.tensor_scalar(zt[:], zt[:], 1.0, 0.0,
                                            OP.min, OP.max)
                    # bounce through ACT so the store DMA (issued on ACT) has
                    # a program-order dep on its producer; its single sem
                    # wait is then the DRAM-tensor WAW vs the previous store.
                    zb = io.tile([P, S], F32, tag="zb" + zr.tensor.name)
                    nc.scalar.copy(zb[:], zt[:])
                    nc.scalar.dma_start(out=zr[t], in_=zb[:])
    nc.finalize()
    return nc


_PROGRAM_CACHE = {}


def _get_program(n_elems, S=96):
    key = (n_elems, S)
    if key not in _PROGRAM_CACHE:
        _PROGRAM_CACHE[key] = build_program(n_elems, S)
    return _PROGRAM_CACHE[key]


def kernel(x_lower, x_upper, elementwise_params):
    x_lower = np.ascontiguousarray(x_lower, dtype=np.float32)
    x_upper = np.ascontiguousarray(x_upper, dtype=np.float32)
    elementwise_params = np.ascontiguousarray(elementwise_params,
                                              dtype=np.float32)
    Bb = x_lower.shape[0]
    per = Bb // N_CORES
    n_elems = per * C * H * W

    nc = _get_program(n_elems)
    in_maps = []
    for c in range(N_CORES):
        sl = slice(c * per, (c + 1) * per)
        in_maps.append({
            "x_lower": x_lower[sl].reshape(n_elems),
            "x_upper": x_upper[sl].reshape(n_elems),
            "elementwise_params": elementwise_params[sl].reshape(n_elems, 34),
        })
    res = run_bass_kernel_spmd(nc, in_maps, list(range(N_CORES)))
    zl = np.concatenate([r["z_lower"].reshape(per, C, H, W)
                         for r in res.results], axis=0)
    zu = np.concatenate([r["z_upper"].reshape(per, C, H, W)
                         for r in res.results], axis=0)
    return zl, zu


if __name__ == "__main__":
    rng = np.random.default_rng(0)
    xl = rng.random((B, C, H, W), dtype=np.float32)
    xu = rng.random((B, C, H, W), dtype=np.float32)
    pp = rng.standard_normal((B, C, H, W, 34), dtype=np.float32)
    zl, zu = kernel(x_lower=xl, x_upper=xu, elementwise_params=pp)
    print("ok", zl.shape, zu.shape, zl.min(), zl.max())
